# revision 1
# baseline (speedup 1.0000x reference)
"""BitLinear (RMSNorm + int8 absmax activation quant + ternary absmean weight
quant + linear + rescale) on 8 Trainium2 NeuronCores.

Sharding: 2 row-groups x 4 col-groups. Each core gets half the rows of x and a
quarter of the weight rows (out_features), computes its [R/2, O/4] output block;
the host assembles the 8 blocks. The global weight scale (mean|W| over the full
weight) is computed on-device with an AllReduce across the 8 cores.

The matmul runs in bf16 which is exact here: quantized activations are integers
in [-127, 127] and quantized weights are in {-1, 0, 1}, both exactly
representable in bf16, and fp32 PSUM accumulation of integer products of this
magnitude is exact.
"""

import sys

sys.path.insert(0, "/opt/trn_rl_repo")

import numpy as np

B, S, D_IN, D_OUT = 4, 2048, 2048, 8192
N_CORES = 8
N_R, N_O = 2, 4
R = B * S // N_R      # rows of x per core
O = D_OUT // N_O      # out cols per core
EPS = 1e-6
MAGIC = 12582912.0    # 1.5 * 2**23: fp32 add/sub round-to-nearest-even trick


def build_nc(rows, d_in, o_cols, n_r, n_o):
    """Build the SPMD bass program for one core."""
    import concourse.tile as tile
    from concourse import bacc, mybir

    f32 = mybir.dt.float32
    bf16 = mybir.dt.bfloat16
    n_cores = n_r * n_o
    P = 128
    n_rt = rows // P            # row tiles
    n_kt = d_in // P            # contraction tiles
    n_ot = o_cols // P          # weight row tiles (out features per core)
    n_p1 = o_cols // 2 // P     # pass-1 tiles over the disjointly-owned half
    nch = min(512, o_cols)      # psum chunk (free dim per matmul)
    n_ch = o_cols // nch        # chunks per row tile
    otpc = nch // P             # o-tiles per chunk
    inv_total = 1.0 / ((n_o * o_cols) * d_in)

    nc = bacc.Bacc("TRN2", target_bir_lowering=False, debug=False,
                   num_devices=n_cores)

    x_d = nc.dram_tensor("x", [rows, d_in], f32, kind="ExternalInput").ap()
    w_d = nc.dram_tensor("w", [o_cols, d_in], f32, kind="ExternalInput").ap()
    # each core's disjoint slice of the full weight, for the |W| mean
    wo_d = nc.dram_tensor("w_own", [o_cols // 2, d_in], f32,
                          kind="ExternalInput").ap()
    g_d = nc.dram_tensor("gamma", [d_in], f32, kind="ExternalInput").ap()
    o_d = nc.dram_tensor("out", [rows, o_cols], f32, kind="ExternalOutput").ap()
    cc_in = nc.dram_tensor("cc_in", [P], f32)
    cc_out = nc.dram_tensor("cc_out", [P], f32, addr_space="Shared")

    with tile.TileContext(nc) as tc:
        with (
            tc.tile_pool(name="xp", bufs=2) as xp,
            tc.tile_pool(name="gp", bufs=2) as gp,
            tc.tile_pool(name="xqp", bufs=2) as xqp,
            tc.tile_pool(name="xqtp", bufs=7) as xqtp,
            tc.tile_pool(name="op", bufs=3) as op,
            tc.tile_pool(name="wp", bufs=2) as wp,
            tc.tile_pool(name="wrp", bufs=3) as wrp,
            tc.tile_pool(name="wqp", bufs=3) as wqp,
            tc.tile_pool(name="wqtp", bufs=1) as wqtp,
            tc.tile_pool(name="gamp", bufs=1) as gamp,
            tc.tile_pool(name="stp", bufs=3) as stp,
            tc.tile_pool(name="cstp", bufs=1) as cstp,
            tc.tile_pool(name="psp", bufs=2, space="PSUM") as psp,
        ):
            # ---- constants / gamma ----
            gam = gamp.tile([P, d_in], f32)
            nc.sync.dma_start(gam[:], g_d.unsqueeze(0).partition_broadcast(P))
            mg = cstp.tile([P, 1], f32)
            nc.vector.memset(mg[:], MAGIC)

            # ---- weight phase 1: partial sum of |w| over the owned slice ----
            asum = cstp.tile([P, n_p1], f32)
            for j in range(n_p1):
                wt = wp.tile([P, d_in], f32, tag="wt")
                nc.sync.dma_start(wt[:], wo_d[j * P:(j + 1) * P, :])
                nc.scalar.activation(wt[:], wt[:],
                                     mybir.ActivationFunctionType.Abs,
                                     accum_out=asum[:, j:j + 1])
            apart = cstp.tile([P, 1], f32)
            nc.vector.reduce_sum(apart[:], asum[:],
                                 axis=mybir.AxisListType.X)
            nc.gpsimd.dma_start(cc_in.ap().unsqueeze(1), apart[:])
            pre_wts = []
            nc.gpsimd.collective_compute(
                "AllReduce", mybir.AluOpType.add,
                replica_groups=[list(range(n_cores))],
                ins=[cc_in.ap()], outs=[cc_out.ap()],
            )
            sums = cstp.tile([P, P], f32)
            nc.gpsimd.dma_start(
                sums[:], cc_out.ap().unsqueeze(0).partition_broadcast(P))
            ws_sum = cstp.tile([P, 1], f32)
            nc.vector.reduce_sum(ws_sum[:], sums[:],
                                 axis=mybir.AxisListType.X)
            w_scale = cstp.tile([P, 1], f32)
            nc.vector.tensor_scalar(w_scale[:], ws_sum[:], inv_total, 1e-5,
                                    op0=mybir.AluOpType.mult,
                                    op1=mybir.AluOpType.max)
            rws = cstp.tile([P, 1], f32)
            nc.vector.reciprocal(rws[:], w_scale[:])
            ws127 = cstp.tile([P, 1], f32)
            nc.vector.tensor_scalar(ws127[:], w_scale[:], 1.0 / 127.0,
                                    None, op0=mybir.AluOpType.mult)

            # ---- weight phase 2: quantize + transpose ----
            # wqT[d_in%128, d_tile, o_tile, o%128] = wq[o, d]
            wqT = wqtp.tile([P, n_kt, n_ot, P], bf16)
            for j in range(n_ot):
                if j < len(pre_wts):
                    wt = pre_wts[j]
                else:
                    wt = wrp.tile([P, d_in], f32, tag="wt2")
                    nc.gpsimd.dma_start(wt[:], w_d[j * P:(j + 1) * P, :])
                nc.scalar.activation(wt[:], wt[:],
                                     mybir.ActivationFunctionType.Identity,
                                     bias=mg[:], scale=rws[:])
                nc.vector.tensor_scalar(wt[:], wt[:], MAGIC, 1.0,
                                        op0=mybir.AluOpType.subtract,
                                        op1=mybir.AluOpType.min)
                wq = wqp.tile([P, d_in], bf16)
                nc.vector.tensor_scalar(wq[:], wt[:], -1.0, None,
                                        op0=mybir.AluOpType.max)
                nc.sync.dma_start_transpose(wqT[:, :, j, :], wq[:])

            # ---- x phase: rmsnorm + quantize + matmul per row tile ----
            for i in range(n_rt):
                xt = xp.tile([P, d_in], f32)
                nc.sync.dma_start(xt[:], x_d[i * P:(i + 1) * P, :])
                gt = gp.tile([P, d_in], f32)
                ss = stp.tile([P, 1], f32, tag="ss")
                # sum of x^2 along the row (gt is a dump buffer here)
                nc.scalar.activation(gt[:], xt[:],
                                     mybir.ActivationFunctionType.Square,
                                     accum_out=ss[:])
                # gt = x * gamma;  mx = max|gt| along the row
                mx = stp.tile([P, 1], f32, tag="mx")
                nc.vector.tensor_tensor(out=gt[:], in0=xt[:], in1=gam[:],
                                        op=mybir.AluOpType.mult)
                nc.vector.tensor_reduce(mx[:], gt[:], axis=mybir.AxisListType.X,
                                        op=mybir.AluOpType.max,
                                        apply_absolute_value=True)
                # x_scale = max(mx / rms, 1e-5); sq = 127/(rms*x_scale)
                t1 = stp.tile([P, 1], f32, tag="t1")
                nc.vector.tensor_scalar(t1[:], ss[:], 1.0 / d_in, EPS,
                                        op0=mybir.AluOpType.mult,
                                        op1=mybir.AluOpType.add)
                rms = stp.tile([P, 1], f32, tag="rms")
                nc.scalar.activation(rms[:], t1[:],
                                     mybir.ActivationFunctionType.Sqrt)
                r1 = stp.tile([P, 1], f32, tag="r1")
                nc.vector.reciprocal(r1[:], rms[:])
                xsc = stp.tile([P, 1], f32, tag="xsc")
                nc.vector.tensor_scalar(xsc[:], mx[:], r1[:], 1e-5,
                                        op0=mybir.AluOpType.mult,
                                        op1=mybir.AluOpType.max)
                d0 = stp.tile([P, 1], f32, tag="d0")
                nc.vector.tensor_tensor(out=d0[:], in0=rms[:], in1=xsc[:],
                                        op=mybir.AluOpType.mult)
                d1 = stp.tile([P, 1], f32, tag="d1")
                nc.vector.tensor_scalar(d1[:], d0[:], 1.0 / 127.0, None,
                                        op0=mybir.AluOpType.mult)
                sq = stp.tile([P, 1], f32, tag="sq")
                nc.vector.reciprocal(sq[:], d1[:])
                osc = stp.tile([P, 1], f32, tag="osc")
                nc.vector.tensor_scalar(osc[:], xsc[:], ws127[:], None,
                                        op0=mybir.AluOpType.mult)
                # xq = round(gt * sq) via magic add/sub, to bf16
                nc.scalar.activation(gt[:], gt[:],
                                     mybir.ActivationFunctionType.Identity,
                                     bias=mg[:], scale=sq[:])
                xq = xqp.tile([P, d_in], bf16)
                nc.vector.tensor_scalar(xq[:], gt[:], MAGIC, None,
                                        op0=mybir.AluOpType.subtract)
                xqT = xqtp.tile([P, n_kt, P], bf16)
                nc.sync.dma_start_transpose(xqT[:], xq[:])
                # matmul: out[r, o] = sum_d xq[r, d] * wq[o, d]
                pss = [psp.tile([P, nch], f32, tag=f"ps{c}", name=f"ps{c}_{i}")
                       for c in range(n_ch)]
                if i < 3:
                    # chunk-outer: lets PE start before all weight tiles are
                    # quantized (chunk c only needs o-tiles [c*otpc, (c+1)*otpc))
                    for c in range(n_ch):
                        for k in range(n_kt):
                            nc.tensor.matmul(
                                pss[c][:], xqT[:, k, :],
                                wqT[:, k, c * otpc:(c + 1) * otpc, :],
                                start=(k == 0), stop=(k == n_kt - 1))
                else:
                    for k in range(n_kt):
                        for c in range(n_ch):
                            nc.tensor.matmul(
                                pss[c][:], xqT[:, k, :],
                                wqT[:, k, c * otpc:(c + 1) * otpc, :],
                                start=(k == 0), stop=(k == n_kt - 1))
                for c in range(n_ch):
                    ot = op.tile([P, nch], f32, tag="oc", name=f"oc_{i}_{c}")
                    nc.scalar.activation(ot[:], pss[c][:],
                                         mybir.ActivationFunctionType.Copy,
                                         scale=osc[:])
                    nc.sync.dma_start(
                        o_d[i * P:(i + 1) * P, c * nch:(c + 1) * nch], ot[:])

    nc.compile()
    return nc


_cache = {}


def _get_nc():
    if "nc" not in _cache:
        _cache["nc"] = build_nc(R, D_IN, O, N_R, N_O)
    return _cache["nc"]


def kernel(x, weight, gamma):
    from concourse.bass_utils import run_bass_kernel_spmd

    nc = _get_nc()
    X = np.ascontiguousarray(np.asarray(x, np.float32).reshape(B * S, D_IN))
    W = np.ascontiguousarray(np.asarray(weight, np.float32))
    G = np.ascontiguousarray(np.asarray(gamma, np.float32))

    in_maps = []
    H = O // 2
    for c in range(N_CORES):
        ri, oj = divmod(c, N_O)
        in_maps.append({
            "x": X[ri * R:(ri + 1) * R],
            "w": W[oj * O:(oj + 1) * O],
            "w_own": W[oj * O + ri * H:oj * O + (ri + 1) * H],
            "gamma": G,
        })
    res = run_bass_kernel_spmd(nc, in_maps, core_ids=list(range(N_CORES)))
    out = np.empty((B * S, D_OUT), np.float32)
    for c in range(N_CORES):
        ri, oj = divmod(c, N_O)
        out[ri * R:(ri + 1) * R, oj * O:(oj + 1) * O] = res.results[c]["out"]
    return out.reshape(B, S, D_OUT)

